# revision 4
# baseline (speedup 1.0000x reference)
"""Multi-head attention (B=2, S=2048, D=1024, H=16, Dk=64) on 8 NeuronCores.

Sharding: 2-way data parallel over batch x 4-way tensor parallel over heads.
Core c handles batch c//4 and heads (c%4)*4 .. (c%4)*4+3, i.e. a 256-column
slice of the QKV projections and the matching 256-row slice of Wo. Each core
computes a partial output projection [S, D]; the host sums the 4 partials per
batch (the all-reduce of the sharding hint) and stacks the batches.

On-core algorithm (matmuls in float32r = full-rate fp32, PSUM accum fp32):
  x^T via PE transpose -> Q^T, K^T head-packed [128, 2, S] (head parity on
  partition halves 0-63/64-127 so the two heads' K=64 score matmuls run
  concurrently in separate PE row groups) and V in natural [t, d'] layout,
  augmented with a ones column -> S^T = K_h Q_h^T -> exp on ACT (1/8 scale
  folded into the activation; no max subtraction needed: scores are O(5)
  for unit-variance inputs, far from fp32 overflow) -> C^T = V_aug^T @
  expS^T where the ones row yields the softmax denominator for free ->
  normalize -> partial out = C^T.T @ Wo_slice + bo/4.
"""
from contextlib import ExitStack

import numpy as np
import concourse.bass as bass
import concourse.mybir as mybir
import concourse.tile as tile
from concourse import bacc
from concourse.bass_utils import run_bass_kernel_spmd
from concourse.masks import make_identity

f32 = mybir.dt.float32
f32r = mybir.dt.float32r
AF = mybir.ActivationFunctionType
ALU = mybir.AluOpType

B, S, D = 2, 2048, 1024
H, DK = 16, 64
NCORES = 8
TP = 4                 # tensor-parallel factor (head groups)
HPC = H // TP          # 4 heads per core
DP = HPC * DK          # 256 = per-core d' slice
SBK = 512              # s-block for attention streaming
NSB = S // SBK         # 4
NT = S // 128          # 16 t-tiles
NDC = D // 128         # 8 contraction chunks over D
NPC = DP // 128        # 2 chunks over d'

_prog_cache = {}


def _build_program():
    nc = bacc.Bacc()
    x = nc.dram_tensor("x", [S, D], f32, kind="ExternalInput")
    wq = nc.dram_tensor("wq", [D, DP], f32, kind="ExternalInput")
    wk = nc.dram_tensor("wk", [D, DP], f32, kind="ExternalInput")
    wv = nc.dram_tensor("wv", [D, DP], f32, kind="ExternalInput")
    wo = nc.dram_tensor("wo", [DP, D], f32, kind="ExternalInput")
    bq = nc.dram_tensor("bq", [DP], f32, kind="ExternalInput")
    bk = nc.dram_tensor("bk", [DP], f32, kind="ExternalInput")
    bv = nc.dram_tensor("bv", [DP], f32, kind="ExternalInput")
    bo4 = nc.dram_tensor("bo4", [D], f32, kind="ExternalInput")
    out = nc.dram_tensor("out", [S, D], f32, kind="ExternalOutput")

    with tile.TileContext(nc) as tc, ExitStack() as top:
        const = top.enter_context(tc.tile_pool(name="const", bufs=1))
        big = top.enter_context(tc.tile_pool(name="big", bufs=1))

        ident = const.tile([128, 128], f32)
        make_identity(nc, ident)

        # weights: DMA fp32 staging, round to f32r once
        wq_r = const.tile([128, NDC, DP], f32r)
        wk_r = const.tile([128, NDC, DP], f32r)
        wv_r = const.tile([128, NDC, DP], f32r)
        wo_r = const.tile([128, NPC, D], f32r)
        with tc.tile_pool(name="stg", bufs=2) as stg:
            for src, dst, npc in ((wq, wq_r, NDC), (wk, wk_r, NDC),
                                  (wv, wv_r, NDC), (wo, wo_r, NPC)):
                sf = stg.tile([128, npc, src.shape[1]], f32, tag="wstg")
                nc.sync.dma_start(out=sf, in_=src.rearrange("(ko ki) d -> ki ko d", ki=128))
                nc.vector.tensor_copy(out=dst, in_=sf)

        bq_sb = const.tile([128, NPC], f32)
        bk_sb = const.tile([128, NPC], f32)
        nc.sync.dma_start(out=bq_sb, in_=bq[:].rearrange("(c p) -> p c", p=128))
        nc.sync.dma_start(out=bk_sb, in_=bk[:].rearrange("(c p) -> p c", p=128))
        bv_1 = const.tile([1, DP], f32)
        nc.sync.dma_start(out=bv_1, in_=bv[:].rearrange("(a d) -> a d", a=1))
        bv_b = const.tile([128, DP], f32)
        nc.gpsimd.partition_broadcast(bv_b, bv_1)
        bo_1 = const.tile([1, D], f32)
        nc.sync.dma_start(out=bo_1, in_=bo4[:].rearrange("(a d) -> a d", a=1))
        bo_b = const.tile([128, D], f32)
        nc.gpsimd.partition_broadcast(bo_b, bo_1)

        # persistent activations
        qt_r = big.tile([128, NPC, S], f32r)
        kt_r = big.tile([128, NPC, S], f32r)
        vaug = big.tile([128, NT, HPC, DK + 1], f32r)
        ct_r = big.tile([128, NPC, S], f32r)

        ones_f = const.tile([128, NT, HPC], f32)
        nc.vector.memset(ones_f, 1.0)
        nc.vector.tensor_copy(out=vaug[:, :, :, DK], in_=ones_f)

        # ---- phase 1: x^T via PE transpose, then QKV projections ----
        with ExitStack() as ph1:
            xtp = ph1.enter_context(tc.tile_pool(name="xt", bufs=1))
            xin = ph1.enter_context(tc.tile_pool(name="xin", bufs=2))
            ps_t = ph1.enter_context(tc.tile_pool(name="ps_t", bufs=2, space="PSUM"))
            ps_p = ph1.enter_context(tc.tile_pool(name="ps_p", bufs=2, space="PSUM"))

            xt_r = xtp.tile([128, NDC, S], f32r)
            for st in range(NT):
                x_t = xin.tile([128, D], f32, tag="x_t")
                nc.sync.dma_start(out=x_t, in_=x[st * 128:(st + 1) * 128, :])
                tp = ps_t.tile([128, NDC * 128], f32, tag="tp")  # 2 banks
                for k in range(NDC):
                    nc.tensor.transpose(
                        out=tp[:, k * 128:(k + 1) * 128],
                        in_=x_t[:, k * 128:(k + 1) * 128],
                        identity=ident,
                    )
                nc.vector.tensor_copy(
                    out=xt_r[:, :, st * 128:(st + 1) * 128],
                    in_=tp.rearrange("p (k s) -> p k s", k=NDC),
                )

            # Q^T / K^T projections (head-packed layout)
            for wr, bias_sb, dst in ((wq_r, bq_sb, qt_r), (wk_r, bk_sb, kt_r)):
                for c in range(NPC):
                    for j in range(NSB):
                        pq = ps_p.tile([128, SBK], f32, tag="pqkv")
                        for k in range(NDC):
                            nc.tensor.matmul(
                                out=pq,
                                lhsT=wr[:, k, c * 128:(c + 1) * 128],
                                rhs=xt_r[:, k, j * SBK:(j + 1) * SBK],
                                start=(k == 0), stop=(k == NDC - 1),
                            )
                        nc.vector.tensor_scalar_add(
                            out=dst[:, c, j * SBK:(j + 1) * SBK],
                            in0=pq, scalar1=bias_sb[:, c:c + 1],
                        )

            # V projection (natural [t, d'] layout into vaug)
            for st in range(NT):
                pv = ps_p.tile([128, DP], f32, tag="pqkv")
                for k in range(NDC):
                    nc.tensor.matmul(
                        out=pv,
                        lhsT=xt_r[:, k, st * 128:(st + 1) * 128],
                        rhs=wv_r[:, k, :],
                        start=(k == 0), stop=(k == NDC - 1),
                    )
                nc.vector.tensor_add(
                    out=vaug[:, st, :, 0:DK],
                    in0=pv.rearrange("p (h d) -> p h d", h=HPC),
                    in1=bv_b.rearrange("p (h d) -> p h d", h=HPC),
                )

        # ---- phase 2: attention + output projection ----
        with ExitStack() as ph2:
            esp = ph2.enter_context(tc.tile_pool(name="esp", bufs=3))
            smal = ph2.enter_context(tc.tile_pool(name="smal", bufs=2))
            outp = ph2.enter_context(tc.tile_pool(name="outp", bufs=3))
            ps_s = ph2.enter_context(tc.tile_pool(name="ps_s", bufs=2, space="PSUM"))
            ps_c = ph2.enter_context(tc.tile_pool(name="ps_c", bufs=1, space="PSUM"))
            ps_o = ph2.enter_context(tc.tile_pool(name="ps_o", bufs=1, space="PSUM"))

            for j in range(NSB):
                for hp in range(NPC):
                    pcs = [ps_c.tile([DK + 1, SBK], f32, tag=f"pc{hh}", name=f"pc{hh}")
                           for hh in range(2)]
                    for t in range(NT):
                        ss = ps_s.tile([128, 2, SBK], f32, tag="ss")
                        for hh in range(2):
                            nc.tensor.matmul(
                                out=ss[:, hh, :],
                                lhsT=kt_r[hh * 64:(hh + 1) * 64, hp, t * 128:(t + 1) * 128],
                                rhs=qt_r[hh * 64:(hh + 1) * 64, hp, j * SBK:(j + 1) * SBK],
                                start=True, stop=True,
                            )
                        es = esp.tile([128, 2, SBK], f32r, tag="es")
                        nc.scalar.activation(out=es, in_=ss, func=AF.Exp, scale=0.125)
                        for hh in range(2):
                            nc.tensor.matmul(
                                out=pcs[hh],
                                lhsT=vaug[:, t, hp * 2 + hh, :],
                                rhs=es[:, hh, :],
                                start=(t == 0), stop=(t == NT - 1),
                            )
                    for hh in range(2):
                        recip = smal.tile([1, SBK], f32, tag="rc")
                        nc.vector.reciprocal(out=recip, in_=pcs[hh][DK:DK + 1, :])
                        rb = smal.tile([64, SBK], f32, tag="rb")
                        nc.gpsimd.partition_broadcast(rb, recip)
                        nc.vector.tensor_mul(
                            out=ct_r[hh * 64:(hh + 1) * 64, hp, j * SBK:(j + 1) * SBK],
                            in0=pcs[hh][0:DK, :],
                            in1=rb,
                        )

                # output projection for this s-block
                for stj in range(SBK // 128):
                    st = j * (SBK // 128) + stj
                    po = ps_o.tile([128, D], f32, tag="po")
                    for c in range(NPC):
                        for nh in range(2):
                            nc.tensor.matmul(
                                out=po[:, nh * 512:(nh + 1) * 512],
                                lhsT=ct_r[:, c, st * 128:(st + 1) * 128],
                                rhs=wo_r[:, c, nh * 512:(nh + 1) * 512],
                                start=(c == 0), stop=(c == NPC - 1),
                            )
                    ob = outp.tile([128, D], f32, tag="ob")
                    nc.vector.tensor_add(out=ob, in0=po, in1=bo_b)
                    nc.sync.dma_start(out=out[st * 128:(st + 1) * 128, :], in_=ob)

    nc.finalize()
    return nc


def _get_program():
    if "nc" not in _prog_cache:
        _prog_cache["nc"] = _build_program()
    return _prog_cache["nc"]


def _make_in_maps(x, Wq, bq, Wk, bk, Wv, bv, Wo, bo):
    in_maps = []
    for c in range(NCORES):
        b, hg = divmod(c, TP)
        sl = slice(hg * DP, (hg + 1) * DP)
        in_maps.append({
            "x": np.ascontiguousarray(x[b]),
            "wq": np.ascontiguousarray(Wq[:, sl]),
            "wk": np.ascontiguousarray(Wk[:, sl]),
            "wv": np.ascontiguousarray(Wv[:, sl]),
            "wo": np.ascontiguousarray(Wo[sl, :]),
            "bq": np.ascontiguousarray(bq[sl]),
            "bk": np.ascontiguousarray(bk[sl]),
            "bv": np.ascontiguousarray(bv[sl]),
            "bo4": np.ascontiguousarray(bo) / np.float32(TP),
        })
    return in_maps


def run(inputs, **spmd_kwargs):
    """Build, run on 8 cores, gather. Returns (output, BassKernelResults)."""
    args = {k: np.asarray(v, dtype=np.float32) for k, v in inputs.items()}
    nc = _get_program()
    in_maps = _make_in_maps(
        args["x"], args["Wq"], args["bq"], args["Wk"], args["bk"],
        args["Wv"], args["bv"], args["Wo"], args["bo"],
    )
    res = run_bass_kernel_spmd(nc, in_maps, list(range(NCORES)), **spmd_kwargs)
    out = np.zeros((B, S, D), dtype=np.float32)
    for c in range(NCORES):
        b = c // TP
        out[b] += res.results[c]["out"]
    return out, res


def kernel(**inputs):
    out, _ = run(inputs)
    return out
